# revision 6
# baseline (speedup 1.0000x reference)
"""Trainium2 Bass kernel for the Lorentz (hyperboloid) embedding loss.

Data-parallel over the batch: B=16384 anchors are sharded 2048-per-core
across 8 NeuronCores. Per anchor the kernel needs the anchor row plus its
50 candidate rows of the 1M x 32 fp32 table. The embedding-row
indirection is resolved on the host into a densely packed per-core
operand (the container's compile path mis-lowers every indirect/gather
DMA primitive).

The packed operand is bf16 with an alpha-transform that keeps the
numerics safe: x0 ~= 1 + 5e-6 would collapse to 1.0 in bf16, so rows are
re-centered. Candidate rows are packed as [x0-1, s_1..s_31], anchor rows
as [1.0, -s_1..s_31]. The elementwise product then satisfies
  sum_d m[d] = beta_k - dot(s_i, s_k)
and  y = d - 1 = alpha_i + beta_k - dot   (+ negligible alpha*beta)
with alpha_i added back from a small fp32 side operand. bf16 halves HBM
traffic and lets the DVE run tensor_tensor at 2x. The d-reduction runs
as a binary tree of tensor_tensor adds (2x) instead of tensor_reduce
(capped at 1x). arcosh is evaluated in y-space:
  t = (1+y) + sqrt((1+y)^2 - 1),  loss = ln(t0 * (sum 1/t + 1e-6)).

Scheduling: the ScalarE square/sqrt of group g run while the DVE works
on group g+1; the DVE-side consumers (t, 1/t, row-sum) are emitted one
group late so the DVE queue never blocks on ScalarE. Activation tables
(3 sets) are preloaded during the DMA ramp by dummy activations.
"""
import os
import sys

for _p in ("/opt/trn_rl_repo", "/root/.axon_site/_ro/trn_rl_repo"):
    if _p not in sys.path and os.path.isdir(_p):
        sys.path.append(_p)

import numpy as np

N_ITEMS_P1 = 1_000_001
DIM = 32
B = 16384
N_KS = 50
W = N_KS + 1          # rows per anchor: anchor + 50 candidates
P = 128               # SBUF partitions = anchors per tile
N_CORES = 8
B_SHARD = B // N_CORES
N_TILES = B_SHARD // P

GRP = 4               # tiles per reduction group
TPI = 2               # tiles per DMA load / multiply instruction
N_GRP = N_TILES // GRP

_nc_cache = None


def _build():
    import concourse.bacc as bacc
    import concourse.tile as tile
    from concourse import mybir

    F32 = mybir.dt.float32
    BF16 = mybir.dt.bfloat16
    AF = mybir.ActivationFunctionType
    OP = mybir.AluOpType

    nc = bacc.Bacc(
        "TRN2", target_bir_lowering=False, debug=False, num_devices=N_CORES
    )
    # g[b, 0, :] = [1, -s_i]; g[b, 1+n, :] = [beta_kn, s_kn]  (host-packed bf16)
    g_in = nc.declare_dram_parameter("g", [B_SHARD, W * DIM], BF16, isOutput=False)
    # alpha[p, t] = x0(anchor t*128+p) - 1, fp32
    a_in = nc.declare_dram_parameter("alpha", [P, N_TILES], F32, isOutput=False)
    loss = nc.declare_dram_parameter("loss", [B_SHARD], F32, isOutput=True)

    from concourse.masks import make_identity

    with tile.TileContext(nc) as tc:
        with (
            tc.tile_pool(name="cons", bufs=1) as cons,
            tc.tile_pool(name="big", bufs=12) as big,
            tc.tile_pool(name="mid", bufs=2) as mid,
            tc.tile_pool(name="small", bufs=2) as small,
            tc.tile_pool(name="psum", bufs=2, space="PSUM") as psum,
        ):
            # issue the first loads before any setup so DMA ramps early
            g_tiles = {}
            n_load = 0
            load_plan = []
            for gi in range(N_GRP):
                load_plan.append(
                    [(0, 1), (1, 1), (2, 2)] if gi == 0
                    else [(0, TPI), (TPI, TPI)]
                )

            def issue_load(gi, tg, tpi):
                nonlocal n_load
                t = gi * GRP + tg
                g = big.tile([P, tpi, W * DIM], BF16, tag="g")
                src = g_in[t * P:(t + tpi) * P, :].rearrange(
                    "(c p) f -> p c f", p=P
                )
                eng = nc.sync if n_load % 2 == 0 else nc.scalar
                eng.dma_start(out=g[:], in_=src)
                n_load += 1
                g_tiles[(gi, tg)] = g

            for tg, tpi in load_plan[0]:
                issue_load(0, tg, tpi)

            alpha_sb = cons.tile([P, N_TILES], F32)
            nc.sync.dma_start(out=alpha_sb[:], in_=a_in[:, :])

            ident = cons.tile([P, P], F32)
            make_identity(nc, ident[:])
            bias_neg1 = cons.tile([P, 1], F32)
            nc.vector.memset(bias_neg1[:], -1.0)
            bias_pos1 = cons.tile([P, 1], F32)
            nc.vector.memset(bias_pos1[:], 1.0)
            # preload the three activation table sets during the DMA ramp
            warm = cons.tile([P, 1], F32)
            nc.scalar.activation(out=warm[:], in_=bias_pos1[:], func=AF.Square,
                                 bias=bias_pos1[:])
            nc.scalar.activation(out=warm[:], in_=bias_pos1[:], func=AF.Sqrt,
                                 bias=bias_pos1[:])
            nc.scalar.activation(out=warm[:], in_=bias_pos1[:], func=AF.Ln)

            ym_all = cons.tile([P, N_TILES, N_KS], F32)
            t_all = cons.tile([P, N_TILES, N_KS], F32)
            s1 = cons.tile([P, N_TILES], F32)
            lv_all = cons.tile([P, N_TILES], F32)

            sq_t = {}
            r_t = {}

            def group_front(gi):
                """DVE work for group gi: multiply, tree-reduce, clamp;
                then kick the ScalarE square/sqrt (consumed one group later)."""
                m = mid.tile([P, GRP, N_KS, DIM], BF16, tag="m")
                for tg, tpi in load_plan[gi]:
                    g = g_tiles.pop((gi, tg))
                    g4 = g[:].rearrange("p c (w d) -> p c w d", d=DIM)
                    nc.vector.tensor_tensor(
                        out=m[:, tg:tg + tpi],
                        in0=g4[:, :, 1:, :],
                        in1=g4[:, :, 0:1, :].to_broadcast([P, tpi, N_KS, DIM]),
                        op=OP.mult,
                    )
                y16 = small.tile([P, GRP, N_KS, 16], BF16, tag="y16")
                nc.vector.tensor_tensor(
                    out=y16[:], in0=m[:, :, :, 0:16], in1=m[:, :, :, 16:32],
                    op=OP.add,
                )
                y8 = small.tile([P, GRP, N_KS, 8], BF16, tag="y8")
                nc.vector.tensor_tensor(
                    out=y8[:], in0=y16[:, :, :, 0:8], in1=y16[:, :, :, 8:16],
                    op=OP.add,
                )
                y4 = small.tile([P, GRP, N_KS, 4], BF16, tag="y4")
                nc.vector.tensor_tensor(
                    out=y4[:], in0=y8[:, :, :, 0:4], in1=y8[:, :, :, 4:8],
                    op=OP.add,
                )
                y2 = small.tile([P, GRP, N_KS, 2], BF16, tag="y2")
                nc.vector.tensor_tensor(
                    out=y2[:], in0=y4[:, :, :, 0:2], in1=y4[:, :, :, 2:4],
                    op=OP.add,
                )
                ys = small.tile([P, GRP, N_KS], F32, tag="ys")
                nc.vector.tensor_tensor(
                    out=ys[:], in0=y2[:, :, :, 0], in1=y2[:, :, :, 1],
                    op=OP.add,
                )
                ya = small.tile([P, GRP, N_KS], F32, tag="ya")
                nc.vector.tensor_tensor(
                    out=ya[:],
                    in0=ys[:],
                    in1=alpha_sb[:, gi * GRP:(gi + 1) * GRP].rearrange(
                        "p (g o) -> p g o", o=1
                    ).to_broadcast([P, GRP, N_KS]),
                    op=OP.add,
                )
                # clamp: reference maps d<=1 -> 1+1e-6, i.e. y<=0 -> 1e-6;
                # max(y, 1e-6) differs only for y in (0, 1e-6): measure-zero.
                ymg = ym_all[:, gi * GRP:(gi + 1) * GRP]
                nc.vector.tensor_scalar(
                    out=ymg, in0=ya[:], scalar1=1e-6, scalar2=None, op0=OP.max
                )
                sq = small.tile([P, GRP, N_KS], F32, tag="sq")
                nc.scalar.activation(
                    out=sq[:], in_=ymg, func=AF.Square, bias=bias_pos1[:]
                )
                r = small.tile([P, GRP, N_KS], F32, tag="r")
                nc.scalar.activation(
                    out=r[:], in_=sq[:], func=AF.Sqrt, bias=bias_neg1[:]
                )
                sq_t[gi] = sq
                r_t[gi] = r

            def group_back(gi):
                """DVE-side tail of group gi: t = (1+ym) + r, 1/t, row-sum."""
                ymg = ym_all[:, gi * GRP:(gi + 1) * GRP]
                tg_ = t_all[:, gi * GRP:(gi + 1) * GRP]
                nc.vector.scalar_tensor_tensor(
                    out=tg_, in0=ymg, scalar=1.0, in1=r_t.pop(gi)[:],
                    op0=OP.add, op1=OP.add,
                )
                rec = small.tile([P, GRP, N_KS], F32, tag="rec")
                nc.vector.reciprocal_approx_fast(out=rec[:].opt(), in_=tg_.opt())
                nc.vector.tensor_reduce(
                    out=s1[:, gi * GRP:(gi + 1) * GRP], in_=rec[:],
                    axis=mybir.AxisListType.X, op=OP.add,
                )

            for gi in range(N_GRP):
                if gi + 1 < N_GRP:
                    for tg, tpi in load_plan[gi + 1]:
                        issue_load(gi + 1, tg, tpi)
                group_front(gi)
                if gi > 0:
                    group_back(gi - 1)
            group_back(N_GRP - 1)

            # loss = ln(t0 * (sum 1/t + 1e-6))
            nc.vector.tensor_scalar(
                out=s1[:], in0=s1[:], scalar1=1e-6, scalar2=None, op0=OP.add
            )
            nc.vector.tensor_tensor(
                out=s1[:], in0=s1[:], in1=t_all[:, :, 0], op=OP.mult
            )
            nc.scalar.activation(out=lv_all[:], in_=s1[:], func=AF.Ln)
            # transpose [128, 16] -> [16, 128] so the store is contiguous
            lv_t_ps = psum.tile([N_TILES, P], F32, space="PSUM")
            nc.tensor.transpose(out=lv_t_ps[:], in_=lv_all[:], identity=ident[:])
            lv_t = cons.tile([N_TILES, P], F32)
            nc.vector.tensor_copy(out=lv_t[:], in_=lv_t_ps[:])
            nc.sync.dma_start(
                out=loss[:].rearrange("(t p) -> t p", p=P), in_=lv_t[:]
            )
    nc.compile()
    return nc


def _get_nc():
    global _nc_cache
    if _nc_cache is None:
        _nc_cache = _build()
    return _nc_cache


def _prep_in_maps(table, I, Ks):
    import ml_dtypes

    table = np.ascontiguousarray(np.asarray(table, dtype=np.float32))
    I = np.asarray(I).astype(np.int64)
    Ks = np.asarray(Ks).astype(np.int64)
    assert table.shape == (N_ITEMS_P1, DIM)
    assert I.shape == (B,) and Ks.shape == (B, N_KS)
    ik = np.concatenate([I[:, None], Ks], axis=1)       # [B, 51]
    rows = table[ik.reshape(-1)].reshape(B, W, DIM)     # [B, 51, 32] fp32
    pack = np.empty((B, W, DIM), dtype=ml_dtypes.bfloat16)
    pack[:, 1:, 0] = rows[:, 1:, 0] - 1.0               # beta_k
    pack[:, 1:, 1:] = rows[:, 1:, 1:]                   # s_k
    pack[:, 0, 0] = 1.0
    pack[:, 0, 1:] = -rows[:, 0, 1:]                    # -s_i
    alpha = (rows[:, 0, 0] - 1.0).astype(np.float32)    # [B]
    g_full = pack.reshape(B, W * DIM)
    in_maps = []
    for c in range(N_CORES):
        sh = np.ascontiguousarray(g_full[c * B_SHARD:(c + 1) * B_SHARD])
        al = np.ascontiguousarray(
            alpha[c * B_SHARD:(c + 1) * B_SHARD].reshape(N_TILES, P).T
        )
        in_maps.append({"g": sh, "alpha": al})
    return in_maps


def _run(table, I, Ks, trace=False, **kwargs):
    from concourse.bass_utils import run_bass_kernel_spmd

    nc = _get_nc()
    in_maps = _prep_in_maps(table, I, Ks)
    res = run_bass_kernel_spmd(
        nc, in_maps, list(range(N_CORES)), trace=trace, **kwargs
    )
    out = np.concatenate(
        [np.asarray(res.results[c]["loss"]) for c in range(N_CORES)]
    ).astype(np.float32)
    return out, res


def kernel(table, I, Ks):
    out, _ = _run(table, I, Ks, trace=False)
    return out
